# revision 4
# baseline (speedup 1.0000x reference)
"""GraphSAGE v7: global dedup + chunked bf16 AllGather + position-sorted
phase-2 gathers with range-sliced dependencies.

Phase 1 is the baseline's (308 indirect gathers -> h1 shard, stored bf16,
all-gathered in 8 chunks). Phase 2's 5632 h1 refs are sorted by h1all
position; each of the 44 indirect gathers passes in_=h1all[:limit] so
Tile's range tracking makes it wait only on the AllGather chunks it
actually reads -- only the last ~2 gathers wait for the final chunk,
shrinking the post-AllGather tail. Aggregation is 2 bf16 matmuls per
gather op against a sparse selector chunk S2 [128, 2*BC] (self/agg
one-hot columns), accumulated in SBUF f32 (per-op start/stop PSUM only),
then the final 2x4 f32 matmuls + ReLU.
"""

import sys

for _p in ("/opt/trn_rl_repo", "/root/.axon_site/_ro/trn_rl_repo"):
    if _p not in sys.path:
        sys.path.insert(0, _p)

import numpy as np
import ml_dtypes

import concourse.bass as bass
import concourse.mybir as mybir
import concourse.tile as tile
from concourse import bacc
from concourse.bass_utils import run_bass_kernel_spmd

N, D, OUT, K = 100000, 256, 128, 10
N1, B = 40960, 4096
NCORES = 8
BC = B // NCORES                 # 512 output rows per core
K1 = K + 1
NREF = BC * K1                   # 5632 phase-2 refs per core
TR = NREF // 128                 # 44 phase-2 gather ops

_CACHE = {}


def _chunk_schedule(sh):
    """AllGather chunks (rows per core): 512s then a tapered tail. Every
    chunk is a multiple of 128, so group boundaries (1024 h1all rows =
    128 shard rows x 8 cores) align with chunk regions."""
    chunks = []
    rem = sh
    while rem > 512:
        chunks.append(512)
        rem -= 512
    if rem > 128:
        chunks.append(rem - 128)
        rem = 128
    chunks.append(rem)
    assert sum(chunks) == sh
    return tuple(chunks)


def _build(SH, LIMITS):
    T1 = SH // 128
    U = SH * NCORES
    CHUNKS = _chunk_schedule(SH)
    CH_START = tuple(sum(CHUNKS[:i]) for i in range(len(CHUNKS)))
    f32 = mybir.dt.float32
    bf16 = mybir.dt.bfloat16
    i32 = mybir.dt.int32
    nc = bacc.Bacc("TRN2", target_bir_lowering=False, debug=False,
                   num_devices=NCORES)
    table = nc.dram_tensor("table", [N, D], f32, kind="ExternalInput").ap()
    ids = nc.dram_tensor("ids", [128, T1 * K1], i32, kind="ExternalInput").ap()
    w1p = nc.dram_tensor("w1p", [2 * D, OUT], f32, kind="ExternalInput").ap()
    w2p = nc.dram_tensor("w2p", [2 * OUT, OUT], f32, kind="ExternalInput").ap()
    ident = nc.dram_tensor("ident", [128, 128], f32, kind="ExternalInput").ap()
    ids2 = nc.dram_tensor("ids2", [128, TR], i32, kind="ExternalInput").ap()
    s2d = nc.dram_tensor("s2d", [TR * 128, 2 * BC], bf16,
                         kind="ExternalInput").ap()
    out = nc.dram_tensor("out", [BC, OUT], f32, kind="ExternalOutput").ap()
    shard = nc.dram_tensor("shard", [SH, OUT], bf16)
    h1all = nc.dram_tensor("h1all", [U, OUT], bf16, addr_space="Shared")

    relu = mybir.ActivationFunctionType.Relu

    with tile.TileContext(nc) as tc:
        with tc.tile_pool(name="const", bufs=1) as constp, \
             tc.tile_pool(name="gat", bufs=6) as gatp, \
             tc.tile_pool(name="agg", bufs=6) as aggp, \
             tc.tile_pool(name="xt", bufs=12) as xtp, \
             tc.tile_pool(name="g2", bufs=8) as g2p, \
             tc.tile_pool(name="s2", bufs=4) as s2p, \
             tc.tile_pool(name="ps", bufs=4, space="PSUM") as psp, \
             tc.tile_pool(name="psh", bufs=2, space="PSUM") as pshp, \
             tc.tile_pool(name="ps2", bufs=1, space="PSUM") as ps2p, \
             tc.tile_pool(name="o", bufs=4) as outp:

            # index tiles load first (first gather only needs ids[:, :K1])
            ids_all = constp.tile([128, T1 * K1], i32, tag="ids_all")
            nc.sync.dma_start(out=ids_all[:, :K1], in_=ids[:, :K1])
            nc.sync.dma_start(out=ids_all[:, K1:], in_=ids[:, K1:])
            idn = constp.tile([128, 128], f32)
            nc.sync.dma_start(out=idn[:], in_=ident[:])
            w1t = constp.tile([128, 4 * OUT], f32, tag="w1")
            for c in range(4):
                nc.sync.dma_start(out=w1t[:, c * OUT:(c + 1) * OUT],
                                  in_=w1p[c * 128:(c + 1) * 128, :])
            w2t = constp.tile([128, 2 * OUT], f32, tag="w2")
            for c in range(2):
                nc.sync.dma_start(out=w2t[:, c * OUT:(c + 1) * OUT],
                                  in_=w2p[c * 128:(c + 1) * 128, :])
            ids2_all = constp.tile([128, TR], i32, tag="ids2_all")
            nc.sync.dma_start(out=ids2_all[:], in_=ids2[:, :])

            # ---- phase 1: compute node-major bf16 h1 shard -> DRAM ----
            for t in range(T1):
                g = gatp.tile([128, K1 * D], f32)
                for k in range(K1):
                    nc.gpsimd.indirect_dma_start(
                        out=g[:, k * D:(k + 1) * D], out_offset=None,
                        in_=table[:],
                        in_offset=bass.IndirectOffsetOnAxis(
                            ap=ids_all[:, t * K1 + k:t * K1 + k + 1], axis=0),
                    )
                a = aggp.tile([128, D], f32)
                nc.vector.tensor_add(a[:], g[:, D:2 * D], g[:, 2 * D:3 * D])
                for k in range(3, K1):
                    nc.vector.tensor_add(a[:], a[:], g[:, k * D:(k + 1) * D])
                srcs = (g[:, 0:128], g[:, 128:256], a[:, 0:128], a[:, 128:256])
                psum_h = pshp.tile([128, 128], f32, space="PSUM")
                for c, src in enumerate(srcs):
                    pt = psp.tile([128, 128], f32, space="PSUM", tag="tp")
                    nc.tensor.transpose(out=pt[:], in_=src, identity=idn[:])
                    xt = xtp.tile([128, 128], f32, tag=f"xt{c}")
                    nc.vector.tensor_copy(out=xt[:], in_=pt[:])
                    nc.tensor.matmul(out=psum_h[:],
                                     lhsT=xt[:],
                                     rhs=w1t[:, c * OUT:(c + 1) * OUT],
                                     start=(c == 0), stop=(c == 3))
                ho = outp.tile([128, OUT], bf16, tag="ho")
                nc.scalar.activation(ho[:], psum_h[:], relu)
                nc.sync.dma_start(out=shard[t * 128:(t + 1) * 128, :],
                                  in_=ho[:])
                done = (t + 1) * 128
                for s, L in zip(CH_START, CHUNKS):
                    if s + L == done:
                        nc.gpsimd.collective_compute(
                            "AllGather", mybir.AluOpType.bypass,
                            replica_groups=[list(range(NCORES))],
                            ins=[shard[s:s + L, :]],
                            outs=[h1all[s * NCORES:(s + L) * NCORES, :]],
                        )

            # ---- phase 2: position-sorted gathers + selector matmuls ----
            accS = constp.tile([128, BC], f32, tag="accS")
            nc.vector.memset(accS[:], 0.0)
            accA = constp.tile([128, BC], f32, tag="accA")
            nc.vector.memset(accA[:], 0.0)
            for i in range(TR):
                gb = g2p.tile([128, OUT], bf16, tag="gb")
                nc.gpsimd.indirect_dma_start(
                    out=gb[:], out_offset=None,
                    in_=h1all[0:LIMITS[i], :],
                    in_offset=bass.IndirectOffsetOnAxis(
                        ap=ids2_all[:, i:i + 1], axis=0))
                s2 = s2p.tile([128, 2 * BC], bf16, tag="s2")
                nc.sync.dma_start(out=s2[:],
                                  in_=s2d[i * 128:(i + 1) * 128, :])
                pS = ps2p.tile([128, BC], f32, space="PSUM", tag="pS")
                nc.tensor.matmul(out=pS[:], lhsT=gb[:], rhs=s2[:, 0:BC],
                                 start=True, stop=True)
                nc.vector.tensor_add(accS[:], accS[:], pS[:])
                pA = ps2p.tile([128, BC], f32, space="PSUM", tag="pA")
                nc.tensor.matmul(out=pA[:], lhsT=gb[:], rhs=s2[:, BC:2 * BC],
                                 start=True, stop=True)
                nc.vector.tensor_add(accA[:], accA[:], pA[:])

            # final layer: per output tile, 2 f32 matmuls + relu
            for t in range(BC // 128):
                ps2 = pshp.tile([128, 128], f32, space="PSUM", tag="psum_h")
                nc.tensor.matmul(out=ps2[:],
                                 lhsT=accS[:, t * 128:(t + 1) * 128],
                                 rhs=w2t[:, 0:OUT], start=True, stop=False)
                nc.tensor.matmul(out=ps2[:],
                                 lhsT=accA[:, t * 128:(t + 1) * 128],
                                 rhs=w2t[:, OUT:2 * OUT],
                                 start=False, stop=True)
                o = outp.tile([128, OUT], f32, tag="o2")
                nc.scalar.activation(o[:], ps2[:], relu)
                nc.sync.dma_start(out=out[t * 128:(t + 1) * 128, :], in_=o[:])

    nc.compile()
    return nc


def _prep_inputs(raw_features, W1, W2, nodes1, neighs1, map2, neighs2):
    raw = np.ascontiguousarray(np.asarray(raw_features, dtype=np.float32))
    W1 = np.asarray(W1, dtype=np.float32)
    W2 = np.asarray(W2, dtype=np.float32)
    nodes1 = np.asarray(nodes1).astype(np.int64)
    neighs1 = np.asarray(neighs1).astype(np.int64)
    map2 = np.asarray(map2).astype(np.int64)
    neighs2 = np.asarray(neighs2).astype(np.int64)

    w1p = np.concatenate([W1[:, :D], W1[:, D:] * (1.0 / K)], axis=1).T
    w2p = np.concatenate([W2[:, :OUT], W2[:, OUT:] * (1.0 / K)], axis=1).T
    w1p = np.ascontiguousarray(w1p, dtype=np.float32)
    w2p = np.ascontiguousarray(w2p, dtype=np.float32)
    ident = np.eye(128, dtype=np.float32)

    # global dedup of layer-1 rows over ALL cores
    refs = np.concatenate([map2, neighs2.reshape(-1)])      # [45056]
    uniq, inv = np.unique(refs, return_inverse=True)
    ua = len(uniq)
    SH = -(-ua // (NCORES * 128)) * 128
    T1 = SH // 128
    U = SH * NCORES
    CHUNKS = _chunk_schedule(SH)
    CH_START = tuple(sum(CHUNKS[:i]) for i in range(len(CHUNKS)))
    uniq_pad = np.concatenate([uniq, np.zeros(U - ua, dtype=uniq.dtype)])
    # position of unique index u in the chunk-interleaved allgather layout
    cidx = np.arange(U) // SH
    r = np.arange(U) % SH
    starts = np.asarray(CH_START)
    sizes = np.asarray(CHUNKS)
    j = np.searchsorted(starts, r, side="right") - 1
    pos_of_u = starts[j] * NCORES + cidx * sizes[j] + (r - starts[j])

    # AllGather region ends in h1all rows (limits rounded up to these)
    region_ends = [(s + L) * NCORES for s, L in zip(CH_START, CHUNKS)]

    percore = []
    raw_limits = np.zeros((NCORES, TR), dtype=np.int64)
    for c in range(NCORES):
        blk = uniq_pad[c * SH:(c + 1) * SH]
        cols = [nodes1[blk]] + [neighs1[blk, k] for k in range(K)]
        idsmat = np.stack(cols, axis=1).astype(np.int32)
        idsmat = np.ascontiguousarray(
            idsmat.reshape(T1, 128, K1).transpose(1, 0, 2).reshape(128, -1))
        # phase-2 refs (position, half, b), sorted by h1all position
        sl = slice(c * BC, (c + 1) * BC)
        self_pos = pos_of_u[inv[np.arange(B)[sl]]]          # [BC]
        neigh_pos = pos_of_u[
            inv[B + np.arange(c * BC * K, (c + 1) * BC * K)]]  # [BC*K]
        pos = np.concatenate([self_pos, neigh_pos])
        half = np.concatenate([np.zeros(BC, np.int64), np.ones(BC * K,
                                                              np.int64)])
        bcol = np.concatenate([np.arange(BC),
                               np.repeat(np.arange(BC), K)])
        order = np.argsort(pos, kind="stable")
        spos, shalf, sb = pos[order], half[order], bcol[order]
        ids2 = np.ascontiguousarray(
            spos.reshape(TR, 128).T.astype(np.int32))       # [128, TR]
        s2m = np.zeros((NREF, 2 * BC), dtype=np.float32)
        np.add.at(s2m, (np.arange(NREF), shalf * BC + sb), 1.0)
        s2m = s2m.astype(ml_dtypes.bfloat16)
        raw_limits[c] = spos.reshape(TR, 128)[:, -1] + 1
        percore.append({"table": raw, "ids": idsmat, "ids2": ids2,
                        "s2d": s2m, "w1p": w1p, "w2p": w2p, "ident": ident})
    lim = raw_limits.max(axis=0)
    LIMITS = tuple(int(next(e for e in region_ends if e >= v)) for v in lim)
    return SH, LIMITS, percore


def run(inputs: dict, trace: bool = False):
    SH, LIMITS, in_maps = _prep_inputs(**inputs)
    key = (SH, LIMITS)
    if key not in _CACHE:
        _CACHE[key] = _build(SH, LIMITS)
    nc = _CACHE[key]
    try:
        res = run_bass_kernel_spmd(nc, in_maps,
                                   core_ids=list(range(NCORES)), trace=trace)
    except Exception:
        res = run_bass_kernel_spmd(nc, in_maps,
                                   core_ids=list(range(NCORES)), trace=trace)
    outp = np.concatenate([res.results[c]["out"] for c in range(NCORES)],
                          axis=0)
    return outp.astype(np.float32), res.exec_time_ns


def kernel(**inputs) -> np.ndarray:
    out, _ = run(inputs, trace=False)
    return out
